# revision 27
# baseline (speedup 1.0000x reference)
"""Trainium2 Bass kernel for nn_LowRankPants (fused per-head linear + gumbel
softmax sampling + grid collapse).

Math (reference):
    factors = einsum('bi,oni->bno', x, W) + b.T          # [B, N, O]
    logits  = factors.reshape(B, O, N)                   # memory reinterpret!
    y       = softmax((logits + gumbel)/tau, axis=-1)
    encoded = einsum('bon,n->bo', y, grid)
    return (encoded, logits)

The reshape(B, O, N) of the contiguous [B, N, O] tensor means
    logits[b, o, n] = x[b] . W[j % O, j // O]   with j = o*N + n.
So with Wall = W.transpose(2,1,0).reshape(I, N*O)  (column j = W[j%O, j//O, :])
the whole thing is a plain GEMM logits_flat = x @ Wall followed by a per-64-bin
softmax. exp((L+g)/tau) = exp(L/tau) * exp(g/tau), and exp(g/tau) is input-only
so it is precomputed on the host -> the kernel's exp reads PSUM directly.

Sharding: tensor-parallel over heads. Core c owns heads [c*32, (c+1)*32) =
flat j columns [c*2048, (c+1)*2048). Each core keeps its [2048, 2048] weight
slice resident in SBUF (as float32r for full-rate fp32 matmul) and streams
batch tiles of 128 rows. Epilogue (softmax + grid collapse) fused on-chip.

Engine budget per [128b, 512j] tile (3.63us of PE): ACT copy+exp ~1.8us,
DVE mul+2 reduces+recip ~2.0us, GpSimd x-conv+grid-mul ~1.3us. PE-bound.
"""

import sys
import types

import numpy as np

if "/opt/trn_rl_repo" not in sys.path:
    sys.path.insert(0, "/opt/trn_rl_repo")

# Hardcoded problem shapes
B, I, O, N = 8192, 2048, 256, 64
TAU = 0.5
NCORES = 8
JL = (O * N) // NCORES          # 2048 j-columns per core
HEADS_L = O // NCORES           # 32 heads per core
P = 128                         # partitions
KS = I // P                     # 16 contraction subtiles
JT = 512                        # j-tile (PSUM bank width, 8 heads x 64 bins)
NJ = JL // JT                   # 4 j tiles per core
NB = B // P                     # 64 batch tiles
HT = JT // N                    # 8 heads per j tile
WARM_B = 8                      # warm-phase batch tiles (j=0 only, W jt0)


def _install_ntff_hook():
    """Register the axon NTFF profile hook if the image's antenv lacks it
    (lets run_bass_kernel_spmd(trace=True) return exec_time_ns)."""
    try:
        import antenv  # noqa: PLC0415

        if "antenv.axon_hooks" not in sys.modules:
            mod = types.ModuleType("antenv.axon_hooks")
            state = {"hook": None}
            mod.set_axon_ntff_profile_hook = lambda h: state.__setitem__("hook", h)
            mod.get_axon_ntff_profile_hook = lambda: state["hook"]
            antenv.axon_hooks = mod
            sys.modules["antenv.axon_hooks"] = mod
        from antenv.axon_hooks import (  # noqa: PLC0415
            get_axon_ntff_profile_hook,
            set_axon_ntff_profile_hook,
        )

        if get_axon_ntff_profile_hook() is None:
            from trn_agent_boot.trn_boot import (  # noqa: PLC0415
                _ntff_profile_via_ctypes,
            )

            hook = _ntff_profile_via_ctypes("/opt/axon/libaxon_pjrt.so")
            if hook is not None:
                set_axon_ntff_profile_hook(hook)
    except Exception:
        pass


def build_bass():
    import concourse.tile as tile
    from concourse import bacc, mybir

    F32 = mybir.dt.float32
    F32R = mybir.dt.float32r
    AF = mybir.ActivationFunctionType
    AX = mybir.AxisListType
    ALU = mybir.AluOpType

    nc = bacc.Bacc("TRN2", target_bir_lowering=False, debug=False,
                   num_devices=NCORES)

    xt = nc.declare_dram_parameter("xt", [I, B], F32, isOutput=False)
    wl = nc.declare_dram_parameter("wl", [I, JL], F32, isOutput=False)
    eg = nc.declare_dram_parameter("eg", [B, JL], F32, isOutput=False)
    gridr = nc.declare_dram_parameter("gridr", [P, JT], F32, isOutput=False)
    logits = nc.declare_dram_parameter("logits", [B, JL], F32, isOutput=True)
    enc = nc.declare_dram_parameter("enc", [B, HEADS_L], F32, isOutput=True)

    # [I, *] viewed with the 128-partition contraction layout: row i = ks*128+p
    xt_v = xt.ap().rearrange("(ks p) b -> p ks b", p=P)       # [128, 16, B]
    wl_v = wl.ap().rearrange("(ks p) j -> p ks j", p=P)       # [128, 16, JL]

    with tile.TileContext(nc) as tc:
        with (
            tc.tile_pool(name="wconst", bufs=1) as wconst,
            tc.tile_pool(name="wstage", bufs=2) as wstage,
            tc.tile_pool(name="consts", bufs=1) as consts,
            tc.tile_pool(name="xstage", bufs=2) as xstage,
            tc.tile_pool(name="xr", bufs=3) as xrp,
            tc.tile_pool(name="egp", bufs=3) as egp,
            tc.tile_pool(name="lgp", bufs=2) as lgp,
            tc.tile_pool(name="ep", bufs=2) as ep,
            tc.tile_pool(name="tp", bufs=2) as tpp,
            tc.tile_pool(name="small", bufs=8) as small,
            tc.tile_pool(name="encp", bufs=2) as encp,
            tc.tile_pool(name="psum", bufs=5, space="PSUM") as psp,
            tc.tile_pool(name="warmps", bufs=1, space="PSUM") as warmps,
        ):
            grid_sb = consts.tile([P, JT], F32)
            nc.sync.dma_start(out=grid_sb, in_=gridr.ap())

            # resident weights, rounded to f32r in place. Chunked [128,1,JT]
            # in jt-major order so the warm phase can start after jt=0.
            w_r = wconst.tile([P, KS, JL], F32R)

            def load_w_chunk(k):
                # full-row 1MB chunks, k-major (matches the j0 chase order);
                # alternate between the two HWDGE rings (SP / Activation) so
                # the W stream uses both FIFOs in parallel during startup
                wstg = wstage.tile([P, JL], F32, name="wstg", tag="wstg")
                eng = nc.sync if (k % 2 == 0) else nc.scalar
                eng.dma_start(out=wstg, in_=wl_v[:, k, :])
                nc.vector.tensor_copy(w_r[:, k, :], wstg[:])

            def start_x(bi):
                b0 = bi * P
                x_stg = xstage.tile([P, KS, P], F32, name="x_stg", tag="x_stg")
                nc.sync.dma_start(out=x_stg, in_=xt_v[:, :, b0:b0 + P])
                x_r = xrp.tile([P, KS, P], F32R, name="x_r", tag="x_r")
                return (x_stg, x_r)

            def convert_x(pair):
                x_stg, x_r = pair
                nc.scalar.activation(
                    x_r[:].rearrange("p a b -> p (a b)"),
                    x_stg[:].rearrange("p a b -> p (a b)"),
                    AF.Copy,
                )

            def load_x(bi):
                pair = start_x(bi)
                convert_x(pair)
                return pair

            def mm_group(x_r, j):
                j0 = j * JT
                psum = psp.tile([P, JT], F32)
                for k in range(KS):
                    nc.tensor.matmul(
                        psum[:],
                        x_r[:, k, :],
                        w_r[:, k, j0:j0 + JT],
                        start=(k == 0),
                        stop=(k == KS - 1),
                    )
                return psum

            def epilogue(psum, bi, j, enc_t):
                b0, j0 = bi * P, j * JT
                # logits out (pre-gumbel)
                lg = lgp.tile([P, JT], F32)
                nc.scalar.activation(lg[:], psum[:], AF.Copy)
                nc.sync.dma_start(
                    out=logits.ap()[b0:b0 + P, j0:j0 + JT], in_=lg[:]
                )
                # e = exp(psum/tau); e2 = e * exp(g/tau)
                e = ep.tile([P, JT], F32)
                nc.scalar.activation(e[:], psum[:], AF.Exp, scale=1.0 / TAU)
                g_t = egp.tile([P, JT], F32)
                nc.sync.dma_start(out=g_t, in_=eg.ap()[b0:b0 + P, j0:j0 + JT])
                e2 = tpp.tile([P, JT], F32, name="e2", tag="e2")
                nc.vector.tensor_mul(e2[:], e[:], g_t[:])
                # s = per-head sum(e2); gsum = per-head sum(e2 * grid)
                s = small.tile([P, HT], F32)
                nc.vector.tensor_reduce(
                    out=s[:], in_=e2[:].rearrange("p (h n) -> p h n", n=N),
                    axis=AX.X, op=ALU.add,
                )
                nc.vector.tensor_mul(e2[:], e2[:], grid_sb[:])
                gsum = small.tile([P, HT], F32)
                nc.vector.tensor_reduce(
                    out=gsum[:], in_=e2[:].rearrange("p (h n) -> p h n", n=N),
                    axis=AX.X, op=ALU.add,
                )
                r = small.tile([P, HT], F32)
                nc.vector.reciprocal(out=r[:], in_=s[:])
                nc.vector.tensor_mul(
                    enc_t[:, j * HT:(j + 1) * HT], gsum[:], r[:]
                )

            # HAM warmer: a dummy f32r weight tile + scratch psum. Small
            # matmuls interleaved with the W load keep the PE clock at 2.4GHz
            # through the DMA-bound startup window.
            dum_w = consts.tile([P, JT], F32R)
            nc.scalar.activation(dum_w[:], grid_sb[:], AF.Copy)
            warm_psum = warmps.tile([P, JT], F32)

            def warm_mm(rhs, n=1):
                for _ in range(n):
                    nc.tensor.matmul(
                        warm_psum[:, :256], dum_w[:, :P], rhs,
                        start=True, stop=True,
                    )

            # x for the first two batch tiles, then resident W. Chunks are
            # h-outer so j-tiles 0..1 are usable after the first 16 chunks.
            # Each chunk conversion feeds a tiny dummy matmul: the dummies
            # spread along the W-load timeline and keep the PE clock (HAM)
            # from re-throttling during the DMA-bound startup.
            x_tiles = {0: load_x(0)}
            warm_mm(dum_w[:, :256], 8)
            for k in range(KS):
                load_w_chunk(k)
            x_tiles[1] = load_x(1)

            for bi in range(0, NB):
                pair = x_tiles.pop(bi)
                enc_t = encp.tile([P, HEADS_L], F32)
                for j in range(NJ):
                    psum = mm_group(pair[1], j)
                    if j == 0 and bi + 2 < NB:
                        # x DMA issued two tiles ahead at j0; the f32r
                        # conversion happens at j2, ~7us later, so the
                        # in-order ACT stream never blocks on the DMA
                        x_tiles[bi + 2] = start_x(bi + 2)
                    if j == 2 and bi + 2 < NB:
                        convert_x(x_tiles[bi + 2])
                    epilogue(psum, bi, j, enc_t)
                nc.sync.dma_start(
                    out=enc.ap()[bi * P:(bi + 1) * P, :], in_=enc_t[:]
                )

    nc.compile()
    return nc


def _prep_inputs(x, W, b, grid, gumbel_noise):
    """Host-side shard prep. Returns per-core in_maps."""
    x = np.asarray(x, dtype=np.float32)
    W = np.asarray(W, dtype=np.float32)
    b = np.asarray(b, dtype=np.float32)
    grid = np.asarray(grid, dtype=np.float32)
    gn = np.asarray(gumbel_noise, dtype=np.float32)

    xt = np.ascontiguousarray(x.T)                               # [I, B]
    wall = np.ascontiguousarray(W.transpose(2, 1, 0)).reshape(I, O * N)
    gridr = np.ascontiguousarray(
        np.broadcast_to(np.tile(grid, HT), (P, JT))
    )
    bias_flat = np.ascontiguousarray(b.T).reshape(O * N)         # [j]
    has_bias = bool(np.any(bias_flat))

    in_maps = []
    for c in range(NCORES):
        j0 = c * JL
        gum_c = np.ascontiguousarray(
            gn[:, c * HEADS_L:(c + 1) * HEADS_L, :]
        ).reshape(B, JL)
        if has_bias:
            gum_c = gum_c + bias_flat[j0:j0 + JL][None, :]
        eg_c = np.exp(gum_c * (1.0 / TAU), dtype=np.float32)
        in_maps.append({
            "xt": xt,
            "wl": np.ascontiguousarray(wall[:, j0:j0 + JL]),
            "eg": eg_c,
            "gridr": gridr,
        })
    return in_maps, b, has_bias


_CACHED_NC = None


def _get_nc():
    global _CACHED_NC
    if _CACHED_NC is None:
        _CACHED_NC = build_bass()
    return _CACHED_NC


def run_sharded(in_maps, trace=False, tmpdir=None):
    from concourse.bass_utils import run_bass_kernel_spmd

    _install_ntff_hook()
    nc = _get_nc()
    return run_bass_kernel_spmd(
        nc, in_maps, list(range(NCORES)), trace=trace, tmpdir=tmpdir
    )


def _assemble(results, b, has_bias):
    encoded = np.empty((B, O), dtype=np.float32)
    logits = np.empty((B, O, N), dtype=np.float32)
    for c in range(NCORES):
        encoded[:, c * HEADS_L:(c + 1) * HEADS_L] = results[c]["enc"]
        logits[:, c * HEADS_L:(c + 1) * HEADS_L, :] = (
            results[c]["logits"].reshape(B, HEADS_L, N)
        )
    if has_bias:
        logits += b[None, :, :]
    return encoded, logits


def kernel(x, W, b, grid, gumbel_noise):
    in_maps, b_arr, has_bias = _prep_inputs(x, W, b, grid, gumbel_noise)
    res = run_sharded(in_maps)
    return _assemble(res.results, b_arr, has_bias)


# revision 28
# speedup vs baseline: 1.1502x; 1.1502x over previous
"""Trainium2 Bass kernel for nn_LowRankPants (fused per-head linear + gumbel
softmax sampling + grid collapse).

Math (reference):
    factors = einsum('bi,oni->bno', x, W) + b.T          # [B, N, O]
    logits  = factors.reshape(B, O, N)                   # memory reinterpret!
    y       = softmax((logits + gumbel)/tau, axis=-1)
    encoded = einsum('bon,n->bo', y, grid)
    return (encoded, logits)

The reshape(B, O, N) of the contiguous [B, N, O] tensor means
    logits[b, o, n] = x[b] . W[j % O, j // O]   with j = o*N + n.
So with Wall = W.transpose(2,1,0).reshape(I, N*O)  (column j = W[j%O, j//O, :])
the whole thing is a plain GEMM logits_flat = x @ Wall followed by a per-64-bin
softmax. exp((L+g)/tau) = exp(L/tau) * exp(g/tau), and exp(g/tau) is input-only
so it is precomputed on the host -> the kernel's exp reads PSUM directly.

Sharding: tensor-parallel over heads. Core c owns heads [c*32, (c+1)*32) =
flat j columns [c*2048, (c+1)*2048). Each core keeps its [2048, 2048] weight
slice resident in SBUF (as float32r for full-rate fp32 matmul) and streams
batch tiles of 128 rows. Epilogue (softmax + grid collapse) fused on-chip.

Engine budget per [128b, 512j] tile (3.63us of PE): ACT copy+exp ~1.8us,
DVE mul+2 reduces+recip ~2.0us, GpSimd x-conv+grid-mul ~1.3us. PE-bound.
"""

import sys
import types

import numpy as np

if "/opt/trn_rl_repo" not in sys.path:
    sys.path.insert(0, "/opt/trn_rl_repo")

# Hardcoded problem shapes
B, I, O, N = 8192, 2048, 256, 64
TAU = 0.5
NCORES = 8
JL = (O * N) // NCORES          # 2048 j-columns per core
HEADS_L = O // NCORES           # 32 heads per core
P = 128                         # partitions
KS = I // P                     # 16 contraction subtiles
JT = 512                        # j-tile (PSUM bank width, 8 heads x 64 bins)
NJ = JL // JT                   # 4 j tiles per core
NB = B // P                     # 64 batch tiles
HT = JT // N                    # 8 heads per j tile
WARM_B = 8                      # warm-phase batch tiles (j=0 only, W jt0)


def _install_ntff_hook():
    """Register the axon NTFF profile hook if the image's antenv lacks it
    (lets run_bass_kernel_spmd(trace=True) return exec_time_ns)."""
    try:
        import antenv  # noqa: PLC0415

        if "antenv.axon_hooks" not in sys.modules:
            mod = types.ModuleType("antenv.axon_hooks")
            state = {"hook": None}
            mod.set_axon_ntff_profile_hook = lambda h: state.__setitem__("hook", h)
            mod.get_axon_ntff_profile_hook = lambda: state["hook"]
            antenv.axon_hooks = mod
            sys.modules["antenv.axon_hooks"] = mod
        from antenv.axon_hooks import (  # noqa: PLC0415
            get_axon_ntff_profile_hook,
            set_axon_ntff_profile_hook,
        )

        if get_axon_ntff_profile_hook() is None:
            from trn_agent_boot.trn_boot import (  # noqa: PLC0415
                _ntff_profile_via_ctypes,
            )

            hook = _ntff_profile_via_ctypes("/opt/axon/libaxon_pjrt.so")
            if hook is not None:
                set_axon_ntff_profile_hook(hook)
    except Exception:
        pass


def build_bass():
    import concourse.tile as tile
    from concourse import bacc, mybir

    F32 = mybir.dt.float32
    F32R = mybir.dt.float32r
    AF = mybir.ActivationFunctionType
    AX = mybir.AxisListType
    ALU = mybir.AluOpType

    nc = bacc.Bacc("TRN2", target_bir_lowering=False, debug=False,
                   num_devices=NCORES)

    xt = nc.declare_dram_parameter("xt", [I, B], F32, isOutput=False)
    wl = nc.declare_dram_parameter("wl", [I, JL], F32, isOutput=False)
    eg = nc.declare_dram_parameter("eg", [B, JL], F32, isOutput=False)
    gridr = nc.declare_dram_parameter("gridr", [P, JT], F32, isOutput=False)
    logits = nc.declare_dram_parameter("logits", [B, JL], F32, isOutput=True)
    enc = nc.declare_dram_parameter("enc", [B, HEADS_L], F32, isOutput=True)

    # [I, *] viewed with the 128-partition contraction layout: row i = ks*128+p
    xt_v = xt.ap().rearrange("(ks p) b -> p ks b", p=P)       # [128, 16, B]
    wl_v = wl.ap().rearrange("(ks p) j -> p ks j", p=P)       # [128, 16, JL]

    with tile.TileContext(nc) as tc:
        with (
            tc.tile_pool(name="wconst", bufs=1) as wconst,
            tc.tile_pool(name="wstage", bufs=2) as wstage,
            tc.tile_pool(name="consts", bufs=1) as consts,
            tc.tile_pool(name="xstage", bufs=2) as xstage,
            tc.tile_pool(name="xr", bufs=3) as xrp,
            tc.tile_pool(name="egp", bufs=3) as egp,
            tc.tile_pool(name="lgp", bufs=2) as lgp,
            tc.tile_pool(name="ep", bufs=2) as ep,
            tc.tile_pool(name="tp", bufs=2) as tpp,
            tc.tile_pool(name="small", bufs=8) as small,
            tc.tile_pool(name="encp", bufs=2) as encp,
            tc.tile_pool(name="psum", bufs=5, space="PSUM") as psp,
            tc.tile_pool(name="warmps", bufs=1, space="PSUM") as warmps,
        ):
            grid_sb = consts.tile([P, JT], F32)
            nc.sync.dma_start(out=grid_sb, in_=gridr.ap())

            # resident weights, rounded to f32r in place. Chunked [128,1,JT]
            # in jt-major order so the warm phase can start after jt=0.
            w_r = wconst.tile([P, KS, JL], F32R)

            def load_w_chunk(k):
                # full-row 1MB chunks, k-major (matches the j0 chase order);
                # alternate between the two HWDGE rings (SP / Activation) so
                # the W stream uses both FIFOs in parallel during startup
                wstg = wstage.tile([P, JL], F32, name="wstg", tag="wstg")
                nc.sync.dma_start(out=wstg, in_=wl_v[:, k, :])
                nc.vector.tensor_copy(w_r[:, k, :], wstg[:])

            def start_x(bi):
                b0 = bi * P
                x_stg = xstage.tile([P, KS, P], F32, name="x_stg", tag="x_stg")
                nc.sync.dma_start(out=x_stg, in_=xt_v[:, :, b0:b0 + P])
                x_r = xrp.tile([P, KS, P], F32R, name="x_r", tag="x_r")
                return (x_stg, x_r)

            def convert_x(pair):
                x_stg, x_r = pair
                nc.scalar.activation(
                    x_r[:].rearrange("p a b -> p (a b)"),
                    x_stg[:].rearrange("p a b -> p (a b)"),
                    AF.Copy,
                )

            def load_x(bi):
                pair = start_x(bi)
                convert_x(pair)
                return pair

            def mm_group(x_r, j):
                j0 = j * JT
                psum = psp.tile([P, JT], F32)
                for k in range(KS):
                    nc.tensor.matmul(
                        psum[:],
                        x_r[:, k, :],
                        w_r[:, k, j0:j0 + JT],
                        start=(k == 0),
                        stop=(k == KS - 1),
                    )
                return psum

            def epilogue(psum, bi, j, enc_t):
                b0, j0 = bi * P, j * JT
                # logits out (pre-gumbel)
                lg = lgp.tile([P, JT], F32)
                nc.scalar.activation(lg[:], psum[:], AF.Copy)
                nc.sync.dma_start(
                    out=logits.ap()[b0:b0 + P, j0:j0 + JT], in_=lg[:]
                )
                # e = exp(psum/tau); e2 = e * exp(g/tau)
                e = ep.tile([P, JT], F32)
                nc.scalar.activation(e[:], psum[:], AF.Exp, scale=1.0 / TAU)
                g_t = egp.tile([P, JT], F32)
                nc.sync.dma_start(out=g_t, in_=eg.ap()[b0:b0 + P, j0:j0 + JT])
                e2 = tpp.tile([P, JT], F32, name="e2", tag="e2")
                nc.vector.tensor_mul(e2[:], e[:], g_t[:])
                # s = per-head sum(e2); gsum = per-head sum(e2 * grid)
                s = small.tile([P, HT], F32)
                nc.vector.tensor_reduce(
                    out=s[:], in_=e2[:].rearrange("p (h n) -> p h n", n=N),
                    axis=AX.X, op=ALU.add,
                )
                nc.vector.tensor_mul(e2[:], e2[:], grid_sb[:])
                gsum = small.tile([P, HT], F32)
                nc.vector.tensor_reduce(
                    out=gsum[:], in_=e2[:].rearrange("p (h n) -> p h n", n=N),
                    axis=AX.X, op=ALU.add,
                )
                r = small.tile([P, HT], F32)
                nc.vector.reciprocal(out=r[:], in_=s[:])
                nc.vector.tensor_mul(
                    enc_t[:, j * HT:(j + 1) * HT], gsum[:], r[:]
                )

            # HAM warmer: a dummy f32r weight tile + scratch psum. Small
            # matmuls interleaved with the W load keep the PE clock at 2.4GHz
            # through the DMA-bound startup window.
            dum_w = consts.tile([P, JT], F32R)
            nc.scalar.activation(dum_w[:], grid_sb[:], AF.Copy)
            warm_psum = warmps.tile([P, JT], F32)

            def warm_mm(rhs, n=1):
                for _ in range(n):
                    nc.tensor.matmul(
                        warm_psum[:, :256], dum_w[:, :P], rhs,
                        start=True, stop=True,
                    )

            # x for the first two batch tiles, then resident W. Chunks are
            # h-outer so j-tiles 0..1 are usable after the first 16 chunks.
            # Each chunk conversion feeds a tiny dummy matmul: the dummies
            # spread along the W-load timeline and keep the PE clock (HAM)
            # from re-throttling during the DMA-bound startup.
            x_tiles = {0: load_x(0)}
            warm_mm(dum_w[:, :256], 8)
            for k in range(KS):
                load_w_chunk(k)
            x_tiles[1] = load_x(1)

            for bi in range(0, NB):
                pair = x_tiles.pop(bi)
                enc_t = encp.tile([P, HEADS_L], F32)
                for j in range(NJ):
                    psum = mm_group(pair[1], j)
                    if j == 0 and bi + 2 < NB:
                        # x DMA issued two tiles ahead at j0; the f32r
                        # conversion happens at j2, ~7us later, so the
                        # in-order ACT stream never blocks on the DMA
                        x_tiles[bi + 2] = start_x(bi + 2)
                    if j == 2 and bi + 2 < NB:
                        convert_x(x_tiles[bi + 2])
                    epilogue(psum, bi, j, enc_t)
                nc.sync.dma_start(
                    out=enc.ap()[bi * P:(bi + 1) * P, :], in_=enc_t[:]
                )

    nc.compile()
    return nc


def _prep_inputs(x, W, b, grid, gumbel_noise):
    """Host-side shard prep. Returns per-core in_maps."""
    x = np.asarray(x, dtype=np.float32)
    W = np.asarray(W, dtype=np.float32)
    b = np.asarray(b, dtype=np.float32)
    grid = np.asarray(grid, dtype=np.float32)
    gn = np.asarray(gumbel_noise, dtype=np.float32)

    xt = np.ascontiguousarray(x.T)                               # [I, B]
    wall = np.ascontiguousarray(W.transpose(2, 1, 0)).reshape(I, O * N)
    gridr = np.ascontiguousarray(
        np.broadcast_to(np.tile(grid, HT), (P, JT))
    )
    bias_flat = np.ascontiguousarray(b.T).reshape(O * N)         # [j]
    has_bias = bool(np.any(bias_flat))

    in_maps = []
    for c in range(NCORES):
        j0 = c * JL
        gum_c = np.ascontiguousarray(
            gn[:, c * HEADS_L:(c + 1) * HEADS_L, :]
        ).reshape(B, JL)
        if has_bias:
            gum_c = gum_c + bias_flat[j0:j0 + JL][None, :]
        eg_c = np.exp(gum_c * (1.0 / TAU), dtype=np.float32)
        in_maps.append({
            "xt": xt,
            "wl": np.ascontiguousarray(wall[:, j0:j0 + JL]),
            "eg": eg_c,
            "gridr": gridr,
        })
    return in_maps, b, has_bias


_CACHED_NC = None


def _get_nc():
    global _CACHED_NC
    if _CACHED_NC is None:
        _CACHED_NC = build_bass()
    return _CACHED_NC


def run_sharded(in_maps, trace=False, tmpdir=None):
    from concourse.bass_utils import run_bass_kernel_spmd

    _install_ntff_hook()
    nc = _get_nc()
    return run_bass_kernel_spmd(
        nc, in_maps, list(range(NCORES)), trace=trace, tmpdir=tmpdir
    )


def _assemble(results, b, has_bias):
    encoded = np.empty((B, O), dtype=np.float32)
    logits = np.empty((B, O, N), dtype=np.float32)
    for c in range(NCORES):
        encoded[:, c * HEADS_L:(c + 1) * HEADS_L] = results[c]["enc"]
        logits[:, c * HEADS_L:(c + 1) * HEADS_L, :] = (
            results[c]["logits"].reshape(B, HEADS_L, N)
        )
    if has_bias:
        logits += b[None, :, :]
    return encoded, logits


def kernel(x, W, b, grid, gumbel_noise):
    in_maps, b_arr, has_bias = _prep_inputs(x, W, b, grid, gumbel_noise)
    res = run_sharded(in_maps)
    return _assemble(res.results, b_arr, has_bias)


# revision 30
# speedup vs baseline: 1.1982x; 1.0417x over previous
"""Trainium2 Bass kernel for nn_LowRankPants (fused per-head linear + gumbel
softmax sampling + grid collapse).

Math (reference):
    factors = einsum('bi,oni->bno', x, W) + b.T          # [B, N, O]
    logits  = factors.reshape(B, O, N)                   # memory reinterpret!
    y       = softmax((logits + gumbel)/tau, axis=-1)
    encoded = einsum('bon,n->bo', y, grid)
    return (encoded, logits)

The reshape(B, O, N) of the contiguous [B, N, O] tensor means
    logits[b, o, n] = x[b] . W[j % O, j // O]   with j = o*N + n.
So with Wall[i, j] = W[j % O, j // O, i]  (= W.transpose(2,1,0).reshape(I, N*O),
since column m = n*O + o of that reshape is W[o, n, :]), the whole thing is a
plain GEMM  logits_flat = x @ Wall  followed by a per-64-bin softmax.

Sharding: tensor-parallel over heads. Core c owns heads [c*32, (c+1)*32) =
flat j columns [c*2048, (c+1)*2048). Each core keeps its [2048, 2048] weight
slice resident in SBUF (as float32r for full-rate fp32 matmul) and streams
batch tiles of 128 rows. The gumbel-softmax + grid collapse epilogue is fused
on-chip (DVE/ACT) per [128 batch, 512 j] PSUM tile.
"""

import sys
import types

import numpy as np

if "/opt/trn_rl_repo" not in sys.path:
    sys.path.insert(0, "/opt/trn_rl_repo")

# Hardcoded problem shapes
B, I, O, N = 8192, 2048, 256, 64
TAU = 0.5
NCORES = 8
JL = (O * N) // NCORES          # 2048 j-columns per core
HEADS_L = O // NCORES           # 32 heads per core
P = 128                         # partitions
KS = I // P                     # 16 contraction subtiles
JT = 512                        # j-tile (PSUM bank width, 8 heads x 64 bins)
NJ = JL // JT                   # 4 j tiles per core
NB = B // P                     # 64 batch tiles


def _install_ntff_hook():
    """Register the axon NTFF profile hook if the image's antenv lacks it
    (lets run_bass_kernel_spmd(trace=True) return exec_time_ns)."""
    try:
        import antenv  # noqa: PLC0415

        if "antenv.axon_hooks" not in sys.modules:
            mod = types.ModuleType("antenv.axon_hooks")
            state = {"hook": None}
            mod.set_axon_ntff_profile_hook = lambda h: state.__setitem__("hook", h)
            mod.get_axon_ntff_profile_hook = lambda: state["hook"]
            antenv.axon_hooks = mod
            sys.modules["antenv.axon_hooks"] = mod
        from antenv.axon_hooks import (  # noqa: PLC0415
            get_axon_ntff_profile_hook,
            set_axon_ntff_profile_hook,
        )

        if get_axon_ntff_profile_hook() is None:
            from trn_agent_boot.trn_boot import (  # noqa: PLC0415
                _ntff_profile_via_ctypes,
            )

            hook = _ntff_profile_via_ctypes("/opt/axon/libaxon_pjrt.so")
            if hook is not None:
                set_axon_ntff_profile_hook(hook)
    except Exception:
        pass


def build_bass():
    import concourse.tile as tile
    from concourse import bacc, mybir

    F32 = mybir.dt.float32
    F32R = mybir.dt.float32r
    AF = mybir.ActivationFunctionType

    nc = bacc.Bacc("TRN2", target_bir_lowering=False, debug=False,
                   num_devices=NCORES)

    xt = nc.declare_dram_parameter("xt", [I, B], F32, isOutput=False)
    wl = nc.declare_dram_parameter("wl", [I, JL], F32, isOutput=False)
    gum = nc.declare_dram_parameter("gum", [B, JL], F32, isOutput=False)
    gridr = nc.declare_dram_parameter("gridr", [P, JT], F32, isOutput=False)
    logits = nc.declare_dram_parameter("logits", [B, JL], F32, isOutput=True)
    enc = nc.declare_dram_parameter("enc", [B, HEADS_L], F32, isOutput=True)

    # [I, *] viewed with the 128-partition contraction layout: row i = ks*128+p
    xt_v = xt.ap().rearrange("(ks p) b -> p ks b", p=P)       # [128, 16, B]
    wl_v = wl.ap().rearrange("(ks p) j -> p ks j", p=P)       # [128, 16, JL]

    with tile.TileContext(nc) as tc:
        with (
            tc.tile_pool(name="wconst", bufs=1) as wconst,
            tc.tile_pool(name="wstage", bufs=2) as wstage,
            tc.tile_pool(name="consts", bufs=1) as consts,
            tc.tile_pool(name="xstage", bufs=2) as xstage,
            tc.tile_pool(name="xr", bufs=2) as xrp,
            tc.tile_pool(name="gumb", bufs=3) as gumb,
            tc.tile_pool(name="lgp", bufs=3) as lgp,
            tc.tile_pool(name="vp", bufs=2) as vp,
            tc.tile_pool(name="ep", bufs=2) as ep,
            tc.tile_pool(name="small", bufs=8) as small,
            tc.tile_pool(name="encp", bufs=2) as encp,
            tc.tile_pool(name="psum", bufs=4, space="PSUM") as psp,
            tc.tile_pool(name="warmps", bufs=1, space="PSUM") as warmps,
        ):
            # grid replica constant
            grid_sb = consts.tile([P, JT], F32)
            nc.sync.dma_start(out=grid_sb, in_=gridr.ap())

            def load_x(bi):
                b0 = bi * P
                x_stg = xstage.tile([P, KS, P], F32, name="x_stg", tag="x_stg")
                nc.sync.dma_start(out=x_stg, in_=xt_v[:, :, b0:b0 + P])
                x_r = xrp.tile([P, KS, P], F32R, name="x_r", tag="x_r")
                nc.scalar.activation(
                    x_r[:].rearrange("p a b -> p (a b)"),
                    x_stg[:].rearrange("p a b -> p (a b)"),
                    AF.Copy,
                )
                return x_r

            # first x tile + a short dummy-matmul burst (warms the PE clock)
            # ahead of the DMA-bound resident-weight load
            dum_w = consts.tile([P, JT], F32R)
            nc.scalar.activation(dum_w[:], grid_sb[:], AF.Copy)
            x_first = load_x(0)
            warm_psum = warmps.tile([P, JT], F32)
            for _ in range(8):
                nc.tensor.matmul(
                    warm_psum[:, :256], dum_w[:, :P], dum_w[:, :256],
                    start=True, stop=True,
                )

            # resident weights, rounded to f32r (staged per k-subtile)
            w_r = wconst.tile([P, KS, JL], F32R)
            for k in range(KS):
                wstg = wstage.tile([P, JL], F32, name="wstg", tag="wstg")
                nc.sync.dma_start(out=wstg, in_=wl_v[:, k, :])
                nc.scalar.activation(w_r[:, k, :], wstg[:], AF.Copy)

            for bi in range(NB):
                b0 = bi * P
                x_r = x_first if bi == 0 else load_x(bi)

                enc_t = encp.tile([P, HEADS_L], F32)

                for j in range(NJ):
                    j0 = j * JT
                    psum = psp.tile([P, JT], F32)
                    for k in range(KS):
                        nc.tensor.matmul(
                            psum[:],
                            x_r[:, k, :],
                            w_r[:, k, j0:j0 + JT],
                            start=(k == 0),
                            stop=(k == KS - 1),
                        )

                    # logits out (pre-gumbel)
                    lg = lgp.tile([P, JT], F32)
                    nc.scalar.activation(lg[:], psum[:], AF.Copy)
                    nc.sync.dma_start(
                        out=logits.ap()[b0:b0 + P, j0:j0 + JT], in_=lg[:]
                    )

                    # e = exp((psum + gumbel) / tau)
                    g_t = gumb.tile([P, JT], F32)
                    nc.sync.dma_start(
                        out=g_t, in_=gum.ap()[b0:b0 + P, j0:j0 + JT]
                    )
                    v = vp.tile([P, JT], F32)
                    nc.vector.tensor_add(v[:], psum[:], g_t[:])
                    e = ep.tile([P, JT], F32)
                    nc.scalar.activation(e[:], v[:], AF.Exp, scale=1.0 / TAU)

                    # s = per-head sum(e); g = per-head sum(e * grid)
                    ev = e[:].rearrange("p (h n) -> p h n", n=N)
                    s = small.tile([P, JT // N], F32)
                    nc.vector.tensor_reduce(
                        out=s[:], in_=ev, axis=mybir.AxisListType.X,
                        op=mybir.AluOpType.add,
                    )
                    nc.vector.tensor_mul(e[:], e[:], grid_sb[:])
                    gsum = small.tile([P, JT // N], F32)
                    nc.vector.tensor_reduce(
                        out=gsum[:], in_=ev, axis=mybir.AxisListType.X,
                        op=mybir.AluOpType.add,
                    )
                    r = small.tile([P, JT // N], F32)
                    nc.vector.reciprocal(out=r[:], in_=s[:])
                    nc.vector.tensor_mul(
                        enc_t[:, j * (JT // N):(j + 1) * (JT // N)],
                        gsum[:], r[:],
                    )

                nc.sync.dma_start(out=enc.ap()[b0:b0 + P, :], in_=enc_t[:])

    nc.compile()
    return nc


def _prep_inputs(x, W, b, grid, gumbel_noise):
    """Host-side shard prep. Returns per-core in_maps."""
    x = np.asarray(x, dtype=np.float32)
    W = np.asarray(W, dtype=np.float32)
    b = np.asarray(b, dtype=np.float32)
    grid = np.asarray(grid, dtype=np.float32)
    gn = np.asarray(gumbel_noise, dtype=np.float32)

    xt = np.ascontiguousarray(x.T)                               # [I, B]
    wall = np.ascontiguousarray(W.transpose(2, 1, 0)).reshape(I, O * N)
    gridr = np.ascontiguousarray(
        np.broadcast_to(np.tile(grid, JT // N), (P, JT))
    )
    bias_flat = np.ascontiguousarray(b.T).reshape(O * N)         # [j]
    has_bias = bool(np.any(bias_flat))

    in_maps = []
    for c in range(NCORES):
        j0 = c * JL
        gum_c = np.ascontiguousarray(
            gn[:, c * HEADS_L:(c + 1) * HEADS_L, :]
        ).reshape(B, JL)
        if has_bias:
            gum_c = gum_c + bias_flat[j0:j0 + JL][None, :]
        in_maps.append({
            "xt": xt,
            "wl": np.ascontiguousarray(wall[:, j0:j0 + JL]),
            "gum": gum_c,
            "gridr": gridr,
        })
    return in_maps, b, has_bias


_CACHED_NC = None


def _get_nc():
    global _CACHED_NC
    if _CACHED_NC is None:
        _CACHED_NC = build_bass()
    return _CACHED_NC


def run_sharded(in_maps, trace=False, tmpdir=None):
    from concourse.bass_utils import run_bass_kernel_spmd

    _install_ntff_hook()
    nc = _get_nc()
    return run_bass_kernel_spmd(
        nc, in_maps, list(range(NCORES)), trace=trace, tmpdir=tmpdir
    )


def _assemble(results, b, has_bias):
    encoded = np.empty((B, O), dtype=np.float32)
    logits = np.empty((B, O, N), dtype=np.float32)
    for c in range(NCORES):
        encoded[:, c * HEADS_L:(c + 1) * HEADS_L] = results[c]["enc"]
        logits[:, c * HEADS_L:(c + 1) * HEADS_L, :] = (
            results[c]["logits"].reshape(B, HEADS_L, N)
        )
    if has_bias:
        logits += b[None, :, :]
    return encoded, logits


def kernel(x, W, b, grid, gumbel_noise):
    in_maps, b_arr, has_bias = _prep_inputs(x, W, b, grid, gumbel_noise)
    res = run_sharded(in_maps)
    return _assemble(res.results, b_arr, has_bias)
